# revision 16
# baseline (speedup 1.0000x reference)
"""Trainium2 Bass kernel for 2-layer edge-MLP GNN with segment-min aggregation.

Strategy (8 NeuronCores, SPMD):
- Shard edges by dst-node bucket: core k owns nodes [12500k, 12500(k+1)).
- Nodes grouped by exact in-degree D, paired into column-groups: stream A =
  partitions 0:64 (hid), stream B = 64:128; a node's D edges occupy D
  consecutive columns.
- Edge columns packed into UNIFORM-DEGREE super-tiles of 1024 columns
  (2 PSUM banks): within a tile every group has the same padded width d_t
  (shorter groups padded with duplicate edges - min unchanged), so
  segment-min is ONE tensor_reduce per super-tile (FD=1024 at 1x on DVE,
  the per-element floor for PSUM reads on TRN2).
- All-fp16 single-pass math (validated vs fp32 reference: rel ~2.9e-3):
  MM1 (K=4: [xA,eA,xB,eB] fp16), ACT Relu+bias -> h fp16, MM2 single
  128x128 block-diag fp16 matmul, fp16 agg, fp16 update MLP.
- Engine balance per 1024-col super-tile (2048 edges): PE ~0.9us
  (4x N=512 matmuls), ACT ~1.0-1.15us (relu evac FD=1024), DVE ~1.2us
  (min-reduce FD=1024). ab2 folded into update-MLP bias on host.
- One compiled program, launched once per layer. Host stages x[src] rows
  (the inter-layer gather) and unpacks agg columns between launches.
"""

import numpy as np

import concourse.bass as bass
import concourse.bacc as bacc
import concourse.mybir as mybir
import concourse.tile as tile
from concourse.bass_utils import run_bass_kernel_spmd

F32 = mybir.dt.float32
F16 = mybir.dt.float16

N_NODES = 100000
N_EDGES = 1600000
N_CORES = 8
NODES_PER_CORE = N_NODES // N_CORES
HID = 64
TILE_N = 1024   # columns per super-tile (= 2 PSUM banks of fp32)
UPD_N = 512     # columns per update-MLP tile (1 PSUM bank)


# ----------------------------------------------------------------------------
# Host-side layout construction (shared compiled structure across cores)
# ----------------------------------------------------------------------------

def build_layout(edge_index):
    """Partition edges by dst core, group nodes by degree, pair A/B streams,
    pack groups into uniform-degree 1024-col super-tiles."""
    src = edge_index[0].astype(np.int64)
    dst = edge_index[1].astype(np.int64)
    deg = np.bincount(dst, minlength=N_NODES)
    maxdeg = int(deg.max())

    # per-core, per-degree node lists
    core_of = dst // NODES_PER_CORE
    nodes_by_core_deg = []  # [core][D] -> list of node ids
    for k in range(N_CORES):
        nd = [[] for _ in range(maxdeg + 1)]
        lo, hi = k * NODES_PER_CORE, (k + 1) * NODES_PER_CORE
        degs_k = deg[lo:hi]
        order = np.argsort(degs_k, kind="stable")
        for i in order:
            d = degs_k[i]
            if d > 0:
                nd[d].append(lo + i)
        nodes_by_core_deg.append(nd)

    # shared group counts per degree class (max over cores, A/B pairing)
    gD = np.zeros(maxdeg + 1, np.int64)
    for d in range(1, maxdeg + 1):
        gD[d] = max((len(nodes_by_core_deg[k][d]) + 1) // 2
                    for k in range(N_CORES))

    # Pack groups into uniform-degree super-tiles: scan classes ascending;
    # a tile's padded width is the max class it contains; capacity check
    # uses the new (wider) width for ALL groups already in the tile.
    tile_meta = []        # per tile: (n_groups, d_pad, agg_col0)
    slots_by_class = {d: [] for d in range(1, maxdeg + 1) if gD[d] > 0}
    cur = []              # [(class_d, count)] in current tile
    cur_n = 0
    agg_col = 0

    def close_tile():
        nonlocal cur, cur_n, agg_col
        if not cur:
            return
        d_pad = max(d for d, _ in cur)
        t = len(tile_meta)
        j = 0
        for d, cnt in cur:
            for _ in range(cnt):
                slots_by_class[d].append((t, j * d_pad, d_pad, agg_col))
                j += 1
                agg_col += 1
        tile_meta.append((j, d_pad, agg_col - j))
        cur, cur_n = [], 0

    for d in range(1, maxdeg + 1):
        g = int(gD[d])
        while g > 0:
            n_fit = TILE_N // d - cur_n
            if n_fit <= 0:
                close_tile()
                continue
            take = min(g, n_fit)
            cur.append((d, take))
            cur_n += take
            g -= take
    close_tile()

    NT = len(tile_meta)
    L = NT * TILE_N
    C = agg_col
    C_pad = ((C + UPD_N - 1) // UPD_N) * UPD_N

    # per-core slot assignment
    order = np.argsort(dst, kind="stable")
    sorted_dst = dst[order]
    starts = np.searchsorted(sorted_dst, np.arange(N_NODES))
    ends = np.searchsorted(sorted_dst, np.arange(N_NODES) + 1)

    slot_edge = np.zeros((N_CORES, 2, L), np.int64)  # edge id per slot
    node_pos = np.full((N_NODES, 2), -1, np.int64)   # node -> (stream, agg_col)
    for k in range(N_CORES):
        nd = nodes_by_core_deg[k]
        any_node = next((n for d in range(1, maxdeg + 1) for n in nd[d]), None)
        assert any_node is not None
        fill_edge = order[starts[any_node]]
        slot_edge[k, :, :] = fill_edge
        for d in range(1, maxdeg + 1):
            if gD[d] == 0:
                continue
            lst = nd[d]
            ng = int(gD[d])
            a_nodes = lst[:ng]
            b_nodes = lst[ng:]
            for j, (t, c0, d_pad, ac) in enumerate(slots_by_class[d]):
                base = t * TILE_N + c0
                for s, nodes in ((0, a_nodes), (1, b_nodes)):
                    if j < len(nodes):
                        n = nodes[j]
                        eids = order[starts[n]:ends[n]]
                        assert len(eids) == d
                        slot_edge[k, s, base:base + d] = eids
                        if d_pad > d:
                            slot_edge[k, s, base + d:base + d_pad] = eids[0]
                        node_pos[n, 0] = s
                        node_pos[n, 1] = ac
                    # else: dummy side keeps the prefilled real edge

    zero_nodes = np.where(deg == 0)[0]
    return dict(
        tile_meta=tile_meta, NT=NT, L=L, C=C, C_pad=C_pad,
        slot_edge=slot_edge, node_pos=node_pos, zero_nodes=zero_nodes,
        src=src, dst=dst,
    )


def build_rhs(layout, x_full, edge_attr):
    """Per-core rhs [5, L] fp16: rows [xA, eA, xB, eB, ones] (ones row
    carries the MM1 bias, so the PSUM->SBUF relu needs no bias operand)."""
    L = layout["L"]
    src = layout["src"]
    rhs = np.zeros((N_CORES, 5, L), np.float16)
    rhs[:, 4, :] = 1.0
    for k in range(N_CORES):
        for s in range(2):
            eids = layout["slot_edge"][k, s]
            rhs[k, 2 * s + 0] = x_full[src[eids]].astype(np.float16)
            rhs[k, 2 * s + 1] = edge_attr[eids].astype(np.float16)
    return rhs


def build_weights(aW1, ab1, aW2, ab2, uW1, ub1, uW2, ub2):
    """Pack one layer's weights for the compiled program (all fp16 except
    biases)."""
    # MM1 lhsT [5, 128]: row 2s+0 -> w1x at cols 64s:64s+64, row 2s+1 -> w1e,
    # row 4 -> ab1 (bias via the ones row of rhs)
    w1 = np.zeros((5, 128), np.float32)
    for s in range(2):
        c0 = 64 * s
        w1[2 * s + 0, c0:c0 + 64] = aW1[0]
        w1[2 * s + 1, c0:c0 + 64] = aW1[1]
        w1[4, c0:c0 + 64] = ab1
    # MM2 lhsT blockdiag [128, 128]
    w2 = np.zeros((128, 128), np.float32)
    for s in range(2):
        w2[64 * s:64 * s + 64, 64 * s:64 * s + 64] = aW2
    # fold ab2 into update bias: ub1' = uW1.T @ ab2 + ub1
    ub1f = (uW1.T @ ab2 + ub1).astype(np.float32)
    ub1vec = np.concatenate([ub1f, ub1f]).reshape(128, 1).astype(np.float32)
    uw1blk = np.zeros((128, 128), np.float32)
    uw1blk[:64, :64] = uW1
    uw1blk[64:, 64:] = uW1
    uw2blk = np.zeros((128, 2), np.float32)
    uw2blk[:64, 0] = uW2[:, 0]
    uw2blk[64:, 1] = uW2[:, 0]
    ub2vec = np.array([[ub2[0]], [ub2[0]]], np.float32)
    return dict(
        w1=w1.astype(np.float16), w2=w2.astype(np.float16),
        ub1vec=ub1vec,
        uw1=uw1blk.astype(np.float16), uw2=uw2blk.astype(np.float16),
        ub2vec=ub2vec,
    )


# ----------------------------------------------------------------------------
# Bass program (compiled once; same structure for all cores and both layers)
# ----------------------------------------------------------------------------

def build_program(layout, bench_reps=1, relu_dve_frac=0.0, chunk=4):
    NT, L, C_pad = layout["NT"], layout["L"], layout["C_pad"]
    tile_meta = layout["tile_meta"]
    NU = C_pad // UPD_N
    CHUNK = chunk  # super-tiles per rhs DMA

    nc = bacc.Bacc("TRN2", target_bir_lowering=False, debug=False,
                   num_devices=N_CORES)
    rhs_d = nc.dram_tensor("rhs", [5, L], F16, kind="ExternalInput")
    w1_d = nc.dram_tensor("w1", [5, 128], F16, kind="ExternalInput")
    w2_d = nc.dram_tensor("w2", [128, 128], F16, kind="ExternalInput")
    ub1_d = nc.dram_tensor("ub1v", [128, 1], F32, kind="ExternalInput")
    uw1_d = nc.dram_tensor("uw1", [128, 128], F16, kind="ExternalInput")
    uw2_d = nc.dram_tensor("uw2", [128, 2], F16, kind="ExternalInput")
    ub2_d = nc.dram_tensor("ub2v", [2, 1], F32, kind="ExternalInput")
    x2_d = nc.dram_tensor("x2out", [2, C_pad], F32, kind="ExternalOutput")

    with tile.TileContext(nc) as tc:
        with (
            tc.tile_pool(name="const", bufs=1) as constp,
            tc.tile_pool(name="stage", bufs=4) as stagep,
            tc.tile_pool(name="hpool", bufs=3) as hp,
            tc.tile_pool(name="aggp", bufs=1) as aggp,
            tc.tile_pool(name="upool", bufs=3) as up,
            tc.tile_pool(name="x2p", bufs=1) as x2p,
            tc.tile_pool(name="prep", bufs=2, space="PSUM") as prep,
            tc.tile_pool(name="msgp", bufs=2, space="PSUM") as msgp,
        ):
            w1_t = constp.tile([5, 128], F16)
            nc.sync.dma_start(w1_t[:], w1_d[:, :])
            w2_t = constp.tile([128, 128], F16)
            nc.sync.dma_start(w2_t[:], w2_d[:, :])
            ub1_t = constp.tile([128, 1], F32)
            nc.sync.dma_start(ub1_t[:], ub1_d[:, :])
            uw1_t = constp.tile([128, 128], F16)
            nc.sync.dma_start(uw1_t[:], uw1_d[:, :])
            uw2_t = constp.tile([128, 2], F16)
            nc.sync.dma_start(uw2_t[:], uw2_d[:, :])
            ub2_t = constp.tile([2, 1], F32)
            nc.sync.dma_start(ub2_t[:], ub2_d[:, :])

            agg_t = aggp.tile([128, C_pad], F16)

            import contextlib
            loop_cm = tc.For_i(0, bench_reps) if bench_reps > 1 \
                else contextlib.nullcontext()
            with loop_cm:
              # ---- edge pipeline, software-pipelined with MM2/reduce TWO
              # super-tiles behind MM1/ACT. With a 1-stage skew the PE queue
              # order (MM1(t), MM2(t-1)) makes ACT(t) transitively wait on
              # ACT(t-1) + MM2 + MM1 + 2 sem hops (~2.3us/super measured).
              # At 2 stages MM2(t-2)'s h dependency is long satisfied, so
              # every engine runs back-to-back (~1.2us/super). ----
              hq = []   # pending (h_tile, (n, d, agg_col0)), oldest first

              def emit_mm2_reduce():
                  h_p, (n_p, d_p, ac_p) = hq.pop(0)
                  msg = msgp.tile([128, TILE_N], F32, tag="msg")
                  nc.tensor.matmul(msg[:, 0:512], w2_t[:], h_p[:, 0:512],
                                   start=True, stop=True)
                  nc.tensor.matmul(msg[:, 512:1024], w2_t[:],
                                   h_p[:, 512:1024],
                                   start=True, stop=True)
                  nc.vector.tensor_reduce(
                      agg_t[:, ac_p:ac_p + n_p],
                      msg[:, 0:n_p * d_p].rearrange("p (n d) -> p n d", d=d_p),
                      axis=mybir.AxisListType.X,
                      op=mybir.AluOpType.min)

              for c0 in range(0, NT, CHUNK):
                  ctiles = min(CHUNK, NT - c0)
                  st = stagep.tile([5, CHUNK * TILE_N], F16, tag="st")
                  nc.sync.dma_start(
                      st[:, :ctiles * TILE_N],
                      rhs_d[:, c0 * TILE_N:(c0 + ctiles) * TILE_N])
                  for j in range(ctiles):
                      t = c0 + j
                      pre = prep.tile([128, TILE_N], F32, tag="pre")
                      nc.tensor.matmul(
                          pre[:, 0:512], w1_t[:],
                          st[:, j * TILE_N:j * TILE_N + 512],
                          start=True, stop=True)
                      nc.tensor.matmul(
                          pre[:, 512:1024], w1_t[:],
                          st[:, j * TILE_N + 512:j * TILE_N + 1024],
                          start=True, stop=True)
                      if len(hq) >= 2:
                          emit_mm2_reduce()
                      h_t = hp.tile([128, TILE_N], F16, tag="h")
                      # bias is folded into MM1 (ones row), so the relu is a
                      # pure max(x, 0) evacuation; split tiles between the
                      # scalar and vector engines to balance load
                      if (t * relu_dve_frac) % 1.0 + relu_dve_frac >= 1.0:
                          nc.vector.tensor_scalar_max(h_t[:], pre[:], 0.0)
                      else:
                          nc.scalar.activation(
                              h_t[:], pre[:],
                              mybir.ActivationFunctionType.Relu,
                              bias=0.0, scale=1.0)
                      hq.append((h_t, tile_meta[t]))
              while hq:
                  emit_mm2_reduce()

              # ---- update MLP (2-stage skew like the edge pipeline) ----
              x2_t = x2p.tile([2, C_pad], F32)
              uq = []  # pending (u_tile, out_col0)

              def emit_u2(tb):
                  u_p, col0 = uq.pop(0)
                  px = msgp.tile([2, UPD_N], F32, tag="msg")
                  nc.tensor.matmul(px[:], uw2_t[:], u_p[:],
                                   start=True, stop=True)
                  nc.vector.tensor_scalar_add(
                      x2_t[:, col0:col0 + UPD_N], px[:], ub2_t[:])

              for t in range(NU):
                  pu = prep.tile([128, UPD_N], F32, tag="pre")
                  nc.tensor.matmul(pu[:], uw1_t[:],
                                   agg_t[:, t * UPD_N:(t + 1) * UPD_N],
                                   start=True, stop=True)
                  if len(uq) >= 2:
                      emit_u2(t)
                  u_t = up.tile([128, UPD_N], F16, tag="u")
                  nc.scalar.activation(u_t[:], pu[:],
                                       mybir.ActivationFunctionType.Relu,
                                       bias=ub1_t[:], scale=1.0)
                  uq.append((u_t, t * UPD_N))
              while uq:
                  emit_u2(NU)
              nc.sync.dma_start(x2_d[:, :], x2_t[:])
    nc.compile()
    return nc


def _update_zero_nodes(x_next, zero_nodes, uW1, ub1, uW2, ub2, ab2):
    if len(zero_nodes) == 0:
        return
    # agg = 0 (+ folded ab2): u = relu(uW1.T @ ab2 + ub1); x = uW2.T u + ub2
    u = np.maximum(uW1.T @ ab2 + ub1, 0.0)
    x_val = float(uW2[:, 0] @ u + ub2[0])
    x_next[zero_nodes] = x_val


def kernel(x, edge_attr, aW1, ab1, aW2, ab2, uW1, ub1, uW2, ub2, edge_index):
    x = np.asarray(x, np.float32)
    edge_attr = np.asarray(edge_attr, np.float32)
    edge_index = np.asarray(edge_index)
    aW1 = np.asarray(aW1, np.float32); ab1 = np.asarray(ab1, np.float32)
    aW2 = np.asarray(aW2, np.float32); ab2 = np.asarray(ab2, np.float32)
    uW1 = np.asarray(uW1, np.float32); ub1 = np.asarray(ub1, np.float32)
    uW2 = np.asarray(uW2, np.float32); ub2 = np.asarray(ub2, np.float32)

    layout = build_layout(edge_index)
    nc = build_program(layout)

    x_cur = x[:, 0].copy()
    ea = edge_attr[:, 0]
    node_pos = layout["node_pos"]
    mapped = node_pos[:, 0] >= 0
    core_of_node = np.arange(N_NODES) // NODES_PER_CORE

    for l in range(2):
        wts = build_weights(aW1[l], ab1[l], aW2[l], ab2[l],
                            uW1[l], ub1[l], uW2[l], ub2[l])
        rhs = build_rhs(layout, x_cur, ea)
        in_maps = []
        for k in range(N_CORES):
            m = {"rhs": np.asarray(rhs[k]),
                 "w1": wts["w1"], "w2": wts["w2"],
                 "ub1v": wts["ub1vec"],
                 "uw1": wts["uw1"], "uw2": wts["uw2"],
                 "ub2v": wts["ub2vec"]}
            in_maps.append(m)
        res = run_bass_kernel_spmd(nc, in_maps, core_ids=list(range(N_CORES)),
                                   trace=False)
        x_next = np.zeros(N_NODES, np.float32)
        for k in range(N_CORES):
            out_k = res.results[k]["x2out"]  # [2, C_pad]
            sel = mapped & (core_of_node == k)
            ids = np.where(sel)[0]
            x_next[ids] = out_k[node_pos[ids, 0], node_pos[ids, 1]]
        _update_zero_nodes(x_next, layout["zero_nodes"],
                           uW1[l], ub1[l], uW2[l], ub2[l], ab2[l])
        x_cur = x_next

    return x_cur[:, None].astype(np.float32)
